# revision 6
# baseline (speedup 1.0000x reference)
"""Inverse STFT (nn_InverseSTFT) as a Bass/Tile kernel on 8 TRN2 NeuronCores.

Math
----
Reference: full spectrum via conjugate symmetry (F = 1024), IDFT per frame,
overlap-add with hop 256, window-sum normalize, trim n_fft//2.

Since hop = N/4, basis[f, 256j+r] = i^(fj) * basis[f, r] exactly, so
  y[256m + r] = (1/N) Re{ sum_f Z[f,m] e^(2*pi*i*f*r/N) },
  Z[f,m] = sum_{j=0..3} i^(fj) X[f, m-j].
Z preserves conjugate symmetry, so y folds to ONE real K=1024 x 256-wide
matmul per output segment (4x less PE work than matmul-per-shift).
Z is computed on the Vector engine as two shifted-add passes:
  Z2[f,c] = X[f,c] + (-1)^f X[f,c-2]
  Z [f,c] = Z2[f,c] + i^f Z2[f,c-1]
Rows are grouped by f mod 4 into 8 chunks of 128 so each pass is a plain
tensor_tensor add/subtract (uniform op per chunk, partition-aligned
Re<->Im partner chunks for odd f) — the only DVE op family with the
2-byte 2x fast mode; scalar_tensor_tensor has none and is 2x slower.
  C0: Re f=4p   C1: [Re512, Im f=4p]  C2: Re f=4p+2  C3: Im f=4p+2
  C4: Re f=4p+1 C5: Im f=4p+1         C6: Re f=4p+3  C7: Im f=4p+3
Chunks are paired into 4 "units" per batch (C0C1 / C2C3 / C4C5 / C6C7);
units with a uniform op run as one merged DVE instruction (chunk-boundary
columns compute garbage that is never read).
Window-sum normalization: basis pre-scaled by 0.25; edge segments fixed up
(m=2: 4/3, m=2000: 4/3, m=2001: 2, m=2002: 4) on the output columns.

Performance notes (from traces):
- DRAM tensors use long contiguous rows; input split into 8 x 1MB units
  alternating across the two HWDGE queues (sync + scalar) so the first
  unit lands early and DVE starts sooner.
- The PE clock starts in a low p-state; a chained warm-up matmul storm on
  junk data during the DMA lead-in ramps it before real work arrives.
- Output DMA'd per 512-column piece right after its PSUM evacuation to
  keep the drain tail short.

Sharding: pure data parallel, 2 batches per core.
"""

import numpy as np
import ml_dtypes

import concourse.bass as bass
import concourse.mybir as mybir
from concourse.tile import TileContext
from concourse import bacc, bass_utils

N_FFT = 1024
HOP = 256
B = 16
T = 2000
NCORES = 8
NB = B // NCORES          # batches per core
TPAD = 2004               # c = t + 1, t in [-1, 2003)
UNITC = 2 * TPAD          # unit = 2 chunks of TPAD cols
OUT_COLS = 2001           # segments m = 2..2002
OUT_PAD = 2048            # padded DRAM row (4 KB bf16)
SC_SIZES = (512, 512, 512, 465)  # psum column chunks over 2001
OUT_LEN = OUT_COLS * HOP  # 512256
N_WARM = 16               # PE p-state warm-up matmuls

F32 = mybir.dt.float32
BF16 = mybir.dt.bfloat16
NP_BF16 = ml_dtypes.bfloat16
ADD = mybir.AluOpType.add
SUB = mybir.AluOpType.subtract


def _row_map():
    rows = []
    rows += [(4 * p, 0) for p in range(128)]                   # C0
    rows += [(512, 0)] + [(4 * p, 1) for p in range(1, 128)]   # C1
    rows += [(4 * p + 2, 0) for p in range(128)]               # C2
    rows += [(4 * p + 2, 1) for p in range(128)]               # C3
    rows += [(4 * p + 1, 0) for p in range(128)]               # C4
    rows += [(4 * p + 1, 1) for p in range(128)]               # C5
    rows += [(4 * p + 3, 0) for p in range(128)]               # C6
    rows += [(4 * p + 3, 1) for p in range(128)]               # C7
    return rows


def _make_basis() -> np.ndarray:
    """[128, 2048] bf16: chunk ch at cols 256*ch, Bz[k, r] with conj-sym
    fold (alpha), 1/N, and the 0.25 steady-state wss normalization."""
    r = np.arange(HOP, dtype=np.float32)
    a32 = np.float32(2.0 * np.pi / N_FFT)
    Bz = np.empty((N_FFT, HOP), np.float32)
    for k, (f, c) in enumerate(_row_map()):
        ang = (np.float32(a32 * np.float32(f)) * r).astype(np.float32)
        alpha = np.float32(1.0 if f in (0, 512) else 2.0)
        v = alpha * np.cos(ang) if c == 0 else -alpha * np.sin(ang)
        Bz[k] = v / np.float32(N_FFT) * np.float32(0.25)
    big = Bz.reshape(8, 128, HOP).transpose(1, 0, 2).reshape(128, 8 * HOP)
    return np.ascontiguousarray(big.astype(NP_BF16))


def _prep_x(stft: np.ndarray) -> np.ndarray:
    """(16,513,2000,2) f32 -> (16, 4, 128, UNITC) bf16 units, zero padded."""
    rows = _row_map()
    F = np.array([f for f, _ in rows])
    C = np.array([c for _, c in rows])
    xt = stft.transpose(0, 3, 1, 2)          # (B, 2, 513, T)
    data = xt[:, C, F, :]                    # (B, 1024, T)
    X = np.zeros((B, N_FFT, TPAD), NP_BF16)
    X[:, :, 1 : 1 + T] = data.astype(NP_BF16)
    X = X.reshape(B, 4, 2, 128, TPAD).transpose(0, 1, 3, 2, 4)
    return np.ascontiguousarray(X.reshape(B, 4, 128, UNITC))


def _build_nc() -> bass.Bass:
    nc = bacc.Bacc()
    x_in = nc.dram_tensor("x_in", [4 * NB, 128, UNITC], BF16, kind="ExternalInput")
    basis_in = nc.dram_tensor("basis_in", [128, 8 * HOP], BF16, kind="ExternalInput")
    out = nc.dram_tensor("out", [NB, 2, 128, OUT_PAD], BF16, kind="ExternalOutput")

    with TileContext(nc) as tc:
        with (
            tc.tile_pool(name="xp", bufs=1) as x_pool,
            tc.tile_pool(name="z2p", bufs=1) as z2_pool,
            tc.tile_pool(name="zp", bufs=1) as z_pool,
            tc.tile_pool(name="bp", bufs=1) as b_pool,
            tc.tile_pool(name="evp", bufs=1) as ev_pool,
            tc.tile_pool(name="ps", bufs=1, space="PSUM") as psum_pool,
        ):
            # basis + warm-up junk via gpsimd (SWDGE/Pool, otherwise idle);
            # x units alternate across the two HWDGE queues (sync, scalar).
            basis_sb = b_pool.tile([128, 8 * HOP], BF16, name="basis_sb", tag="basis")
            nc.gpsimd.dma_start(basis_sb[:, :], basis_in[:, :])
            junk = b_pool.tile([128, 512], BF16, name="junk", tag="junk")
            nc.gpsimd.memset(junk[:, :], 0.0)

            # PE p-state warm-up: chained junk matmuls, done before real work.
            ps_w = psum_pool.tile([128, 512], F32, name="psw", tag="ps3_1")
            for w in range(N_WARM):
                nc.tensor.matmul(ps_w[:, :], junk[:, :128], junk[:, :],
                                 start=(w == 0), stop=(w == N_WARM - 1))

            # all x units stream on the single sync HWDGE queue (splitting
            # across queues measured ~15% slower aggregate input)
            x_sb = {}
            for b in range(NB):
                for u in range(4):
                    xt = x_pool.tile([128, UNITC], BF16, name=f"x{b}_{u}",
                                     tag=f"x{b}_{u}")
                    nc.sync.dma_start(xt[:, :], x_in[4 * b + u])
                    x_sb[b, u] = xt

            # DVE processes each unit to completion (z2 then z) so the PE can
            # consume it immediately; z2 of u1 goes to the otherwise-idle
            # GpSimd engine (rate probe for future rebalancing).
            z_sb = {}
            for b in range(NB):
                for u in range(4):
                    z2t = z2_pool.tile([128, UNITC], BF16, name=f"z2_{u}",
                                       tag=f"z2_{u}")
                    xt = x_sb[b, u]
                    eng = nc.gpsimd if u == 1 else nc.vector
                    # merged Z2 over both chunks; boundary cols unread
                    eng.tensor_tensor(
                        out=z2t[:, 2:UNITC],
                        in0=xt[:, 2:UNITC],
                        in1=xt[:, 0 : UNITC - 2],
                        op=ADD if u < 2 else SUB,
                    )
                    zt = z_pool.tile([128, UNITC], BF16, name=f"z{b}_{u}",
                                     tag=f"z{b}_{u}")
                    if u < 2:
                        # C0C1: Z = Z2[c] + Z2[c-1]; C2C3: minus. Merged:
                        # shift-by-1 stays chunk-aligned across the unit.
                        nc.vector.tensor_tensor(
                            out=zt[:, 3:UNITC],
                            in0=z2t[:, 3:UNITC],
                            in1=z2t[:, 2 : UNITC - 1],
                            op=ADD if u == 0 else SUB,
                        )
                    else:
                        # odd f: partner chunk Re<->Im within the unit
                        reo, imo = 0, TPAD
                        op_re, op_im = (SUB, ADD) if u == 2 else (ADD, SUB)
                        nc.vector.tensor_tensor(
                            out=zt[:, reo + 3 : reo + TPAD],
                            in0=z2t[:, reo + 3 : reo + TPAD],
                            in1=z2t[:, imo + 2 : imo + TPAD - 1],
                            op=op_re,
                        )
                        nc.vector.tensor_tensor(
                            out=zt[:, imo + 3 : imo + TPAD],
                            in0=z2t[:, imo + 3 : imo + TPAD],
                            in1=z2t[:, reo + 2 : reo + TPAD - 1],
                            op=op_im,
                        )
                    z_sb[b, u] = zt

            for b in range(NB):
                ev = {}
                for h in range(2):
                    ev[h] = ev_pool.tile([128, OUT_PAD], BF16, name=f"ev{b}_{h}",
                                         tag=f"ev{b}_{h}")
                # arrival-interleaved matmul emission: each unit's 2 chunks
                # contribute to all 8 PSUM groups as soon as its Z is ready,
                # so the PE streams behind the DVE instead of waiting for the
                # whole batch; only the last unit's matmuls are tail work.
                ps = {}
                for u in range(4):
                    for sc in range(4):
                        cols = SC_SIZES[sc]
                        for h in range(2):
                            if u == 0:
                                ps[sc, h] = psum_pool.tile(
                                    [128, 512], F32, name="ps", tag=f"ps{sc}_{h}")
                            for i in range(2):
                                ch = 2 * u + i
                                o = TPAD * i + 3 + 512 * sc
                                nc.tensor.matmul(
                                    ps[sc, h][:, :cols],
                                    basis_sb[:, HOP * ch + 128 * h : HOP * ch + 128 * h + 128],
                                    z_sb[b, u][:, o : o + cols],
                                    start=(ch == 0),
                                    stop=(ch == 7),
                                )
                for sc in range(4):
                    cols = SC_SIZES[sc]
                    for h in range(2):
                        evt = ev[h]
                        nc.scalar.copy(evt[:, 512 * sc : 512 * sc + cols],
                                       ps[sc, h][:, :cols])
                        if sc == 0:  # m=2 has 3 frames
                            nc.scalar.mul(evt[:, 0:1], ps[sc, h][:, 0:1], 4.0 / 3.0)
                        elif sc == 3:  # m=2000,2001,2002
                            nc.scalar.mul(evt[:, 1998:1999], ps[sc, h][:, 462:463], 4.0 / 3.0)
                            nc.scalar.mul(evt[:, 1999:2000], ps[sc, h][:, 463:464], 2.0)
                            nc.scalar.mul(evt[:, 2000:2001], ps[sc, h][:, 464:465], 4.0)
                        nc.scalar.dma_start(
                            out[b, h, :, 512 * sc : 512 * sc + cols],
                            evt[:, 512 * sc : 512 * sc + cols],
                        )
    nc.finalize()
    return nc


def _run(inputs: dict, trace: bool = False):
    stft = np.asarray(inputs["stft_matrix"], dtype=np.float32)
    X = _prep_x(stft)                        # (16, 4, 128, UNITC)
    basis = _make_basis()
    in_maps = [
        {
            "x_in": np.ascontiguousarray(
                X[NB * c : NB * (c + 1)].reshape(4 * NB, 128, UNITC)
            ),
            "basis_in": basis,
        }
        for c in range(NCORES)
    ]
    nc = _build_nc()
    res = bass_utils.run_bass_kernel_spmd(
        nc, in_maps, core_ids=list(range(NCORES)), trace=trace
    )
    outs = []
    for c in range(NCORES):
        o = np.asarray(res.results[c]["out"])  # (NB, 2, 128, OUT_PAD) bf16
        o = o.reshape(NB, 2 * 128, OUT_PAD)[:, :, :OUT_COLS].astype(np.float32)
        outs.append(np.ascontiguousarray(o.transpose(0, 2, 1)).reshape(NB, OUT_LEN))
    return np.concatenate(outs, axis=0), res


def kernel(**inputs) -> np.ndarray:
    out, _ = _run(inputs, trace=False)
    return out


# revision 10
# speedup vs baseline: 1.2113x; 1.2113x over previous
"""Inverse STFT (nn_InverseSTFT) as a Bass/Tile kernel on 8 TRN2 NeuronCores.

Math
----
Reference: full spectrum via conjugate symmetry (F = 1024), IDFT per frame,
overlap-add with hop 256, window-sum normalize, trim n_fft//2.

Since hop = N/4, basis[f, 256j+r] = i^(fj) * basis[f, r] exactly, so
  y[256m + r] = (1/N) Re{ sum_f Z[f,m] e^(2*pi*i*f*r/N) },
  Z[f,m] = sum_{j=0..3} i^(fj) X[f, m-j].
Z preserves conjugate symmetry, so y folds to ONE real K=1024 x 256-wide
matmul per output segment (4x less PE work than matmul-per-shift).
Z is computed on the Vector engine as two shifted-add passes:
  Z2[f,c] = X[f,c] + (-1)^f X[f,c-2]
  Z [f,c] = Z2[f,c] + i^f Z2[f,c-1]
Rows are grouped by f mod 4 into 8 chunks of 128 so each pass is a plain
tensor_tensor add/subtract (uniform op per chunk, partition-aligned
Re<->Im partner chunks for odd f) — the only DVE op family with the
2-byte 2x fast mode; scalar_tensor_tensor has none and is 2x slower.
  C0: Re f=4p   C1: [Re512, Im f=4p]  C2: Re f=4p+2  C3: Im f=4p+2
  C4: Re f=4p+1 C5: Im f=4p+1         C6: Re f=4p+3  C7: Im f=4p+3
Chunks are paired into 4 "units" per batch (C0C1 / C2C3 / C4C5 / C6C7);
units with a uniform op run as one merged DVE instruction (chunk-boundary
columns compute garbage that is never read).
Window-sum normalization: basis pre-scaled by 0.25; edge segments fixed up
(m=2: 4/3, m=2000: 4/3, m=2001: 2, m=2002: 4) on the output columns.

Performance notes (from traces):
- DRAM tensors use long contiguous rows; input split into 8 x 1MB units
  alternating across the two HWDGE queues (sync + scalar) so the first
  unit lands early and DVE starts sooner.
- The PE clock starts in a low p-state; a chained warm-up matmul storm on
  junk data during the DMA lead-in ramps it before real work arrives.
- Output DMA'd per 512-column piece right after its PSUM evacuation to
  keep the drain tail short.

Sharding: pure data parallel, 2 batches per core.
"""

import numpy as np
import ml_dtypes

import concourse.bass as bass
import concourse.mybir as mybir
from concourse.tile import TileContext
from concourse import bacc, bass_utils

N_FFT = 1024
HOP = 256
B = 16
T = 2000
NCORES = 8
NB = B // NCORES          # batches per core
TPAD = 2004               # c = t + 1, t in [-1, 2003)
UNITC = 2 * TPAD          # unit = 2 chunks of TPAD cols
OUT_COLS = 2001           # segments m = 2..2002
OUT_PAD = 2048            # padded DRAM row (4 KB bf16)
SC_SIZES = (512, 512, 512, 465)  # psum column chunks over 2001
OUT_LEN = OUT_COLS * HOP  # 512256
N_WARM = 26               # PE p-state warm-up matmuls

F32 = mybir.dt.float32
BF16 = mybir.dt.bfloat16
NP_BF16 = ml_dtypes.bfloat16
ADD = mybir.AluOpType.add
SUB = mybir.AluOpType.subtract


def _row_map():
    rows = []
    rows += [(4 * p, 0) for p in range(128)]                   # C0
    rows += [(512, 0)] + [(4 * p, 1) for p in range(1, 128)]   # C1
    rows += [(4 * p + 2, 0) for p in range(128)]               # C2
    rows += [(4 * p + 2, 1) for p in range(128)]               # C3
    rows += [(4 * p + 1, 0) for p in range(128)]               # C4
    rows += [(4 * p + 1, 1) for p in range(128)]               # C5
    rows += [(4 * p + 3, 0) for p in range(128)]               # C6
    rows += [(4 * p + 3, 1) for p in range(128)]               # C7
    return rows


def _make_basis() -> np.ndarray:
    """[128, 2048] bf16: chunk ch at cols 256*ch, Bz[k, r] with conj-sym
    fold (alpha), 1/N, and the 0.25 steady-state wss normalization."""
    r = np.arange(HOP, dtype=np.float32)
    a32 = np.float32(2.0 * np.pi / N_FFT)
    Bz = np.empty((N_FFT, HOP), np.float32)
    for k, (f, c) in enumerate(_row_map()):
        ang = (np.float32(a32 * np.float32(f)) * r).astype(np.float32)
        alpha = np.float32(1.0 if f in (0, 512) else 2.0)
        v = alpha * np.cos(ang) if c == 0 else -alpha * np.sin(ang)
        Bz[k] = v / np.float32(N_FFT) * np.float32(0.25)
    big = Bz.reshape(8, 128, HOP).transpose(1, 0, 2).reshape(128, 8 * HOP)
    return np.ascontiguousarray(big.astype(NP_BF16))


def _prep_x(stft: np.ndarray) -> np.ndarray:
    """(16,513,2000,2) f32 -> (16, 4, 128, UNITC) bf16 units, zero padded."""
    rows = _row_map()
    F = np.array([f for f, _ in rows])
    C = np.array([c for _, c in rows])
    xt = stft.transpose(0, 3, 1, 2)          # (B, 2, 513, T)
    data = xt[:, C, F, :]                    # (B, 1024, T)
    X = np.zeros((B, N_FFT, TPAD), NP_BF16)
    X[:, :, 1 : 1 + T] = data.astype(NP_BF16)
    X = X.reshape(B, 4, 2, 128, TPAD).transpose(0, 1, 3, 2, 4)
    return np.ascontiguousarray(X.reshape(B, 4, 128, UNITC))


def _build_nc() -> bass.Bass:
    nc = bacc.Bacc()
    x_in = nc.dram_tensor("x_in", [4 * NB, 128, UNITC], BF16, kind="ExternalInput")
    basis_in = nc.dram_tensor("basis_in", [128, 8 * HOP], BF16, kind="ExternalInput")
    out = nc.dram_tensor("out", [NB, 2, 128, OUT_PAD], BF16, kind="ExternalOutput")

    with TileContext(nc) as tc:
        with (
            tc.tile_pool(name="xp", bufs=1) as x_pool,
            tc.tile_pool(name="z2p", bufs=1) as z2_pool,
            tc.tile_pool(name="zp", bufs=1) as z_pool,
            tc.tile_pool(name="bp", bufs=1) as b_pool,
            tc.tile_pool(name="evp", bufs=1) as ev_pool,
            tc.tile_pool(name="ps", bufs=1, space="PSUM") as psum_pool,
        ):
            # basis + warm-up junk via gpsimd (SWDGE/Pool, otherwise idle);
            # x units alternate across the two HWDGE queues (sync, scalar).
            basis_sb = b_pool.tile([128, 8 * HOP], BF16, name="basis_sb", tag="basis")
            nc.gpsimd.dma_start(basis_sb[:, :], basis_in[:, :])
            junk = b_pool.tile([128, 512], BF16, name="junk", tag="junk")
            nc.gpsimd.memset(junk[:, :], 0.0)

            # PE p-state warm-up: chained junk matmuls, done before real work.
            ps_w = psum_pool.tile([128, 512], F32, name="psw", tag="ps3_1")
            for w in range(N_WARM):
                nc.tensor.matmul(ps_w[:, :], junk[:, :128], junk[:, :],
                                 start=(w == 0), stop=(w == N_WARM - 1))

            # all x units stream on the single sync HWDGE queue (splitting
            # across queues measured ~15% slower aggregate input); the very
            # first unit is split into 2 chunk DMAs so DVE starts earlier.
            x_sb = {}
            for b in range(NB):
                for u in range(4):
                    xt = x_pool.tile([128, UNITC], BF16, name=f"x{b}_{u}",
                                     tag=f"x{b}_{u}")
                    if b == 0 and u == 0:
                        nc.sync.dma_start(xt[:, :TPAD], x_in[0, :, :TPAD])
                        nc.sync.dma_start(xt[:, TPAD:], x_in[0, :, TPAD:])
                    else:
                        nc.sync.dma_start(xt[:, :], x_in[4 * b + u])
                    x_sb[b, u] = xt

            # DVE processes each unit to completion (z2 then z) so the PE can
            # consume it immediately.
            z_sb = {}
            for b in range(NB):
                for u in range(4):
                    z2t = z2_pool.tile([128, UNITC], BF16, name=f"z2_{u}",
                                       tag=f"z2_{u}")
                    xt = x_sb[b, u]
                    if b == 0 and u == 0:
                        # split per chunk to start right after each half-DMA
                        # (slice-level deps), shaving pipeline lead-in.
                        for i in range(2):
                            o = TPAD * i
                            nc.vector.tensor_tensor(
                                out=z2t[:, o + 2 : o + TPAD],
                                in0=xt[:, o + 2 : o + TPAD],
                                in1=xt[:, o : o + TPAD - 2],
                                op=ADD,
                            )
                    else:
                        # merged Z2 over both chunks; boundary cols unread
                        nc.vector.tensor_tensor(
                            out=z2t[:, 2:UNITC],
                            in0=xt[:, 2:UNITC],
                            in1=xt[:, 0 : UNITC - 2],
                            op=ADD if u < 2 else SUB,
                        )
                    zt = z_pool.tile([128, UNITC], BF16, name=f"z{b}_{u}",
                                     tag=f"z{b}_{u}")
                    if u < 2:
                        # C0C1: Z = Z2[c] + Z2[c-1]; C2C3: minus. Merged:
                        # shift-by-1 stays chunk-aligned across the unit.
                        nc.vector.tensor_tensor(
                            out=zt[:, 3:UNITC],
                            in0=z2t[:, 3:UNITC],
                            in1=z2t[:, 2 : UNITC - 1],
                            op=ADD if u == 0 else SUB,
                        )
                    else:
                        # odd f: partner chunk Re<->Im within the unit
                        reo, imo = 0, TPAD
                        op_re, op_im = (SUB, ADD) if u == 2 else (ADD, SUB)
                        nc.vector.tensor_tensor(
                            out=zt[:, reo + 3 : reo + TPAD],
                            in0=z2t[:, reo + 3 : reo + TPAD],
                            in1=z2t[:, imo + 2 : imo + TPAD - 1],
                            op=op_re,
                        )
                        nc.vector.tensor_tensor(
                            out=zt[:, imo + 3 : imo + TPAD],
                            in0=z2t[:, imo + 3 : imo + TPAD],
                            in1=z2t[:, reo + 2 : reo + TPAD - 1],
                            op=op_im,
                        )
                    z_sb[b, u] = zt

            for b in range(NB):
                ev = {}
                for h in range(2):
                    ev[h] = ev_pool.tile([128, OUT_PAD], BF16, name=f"ev{b}_{h}",
                                         tag=f"ev{b}_{h}")
                # arrival-interleaved matmul emission: each unit's 2 chunks
                # contribute to all 8 PSUM groups as soon as its Z is ready,
                # so the PE streams behind the DVE instead of waiting for the
                # whole batch. In the last unit's block each group closes
                # (stop) and is immediately evacuated + DMA'd, so the drain
                # tail is one group, not eight.
                ps = {}
                for u in range(4):
                    for sc in range(4):
                        cols = SC_SIZES[sc]
                        for h in range(2):
                            if u == 0:
                                ps[sc, h] = psum_pool.tile(
                                    [128, 512], F32, name="ps", tag=f"ps{sc}_{h}")
                            for i in range(2):
                                ch = 2 * u + i
                                o = TPAD * i + 3 + 512 * sc
                                nc.tensor.matmul(
                                    ps[sc, h][:, :cols],
                                    basis_sb[:, HOP * ch + 128 * h : HOP * ch + 128 * h + 128],
                                    z_sb[b, u][:, o : o + cols],
                                    start=(ch == 0),
                                    stop=(ch == 7),
                                )
                            if u < 3:
                                continue
                            evt = ev[h]
                            nc.scalar.copy(evt[:, 512 * sc : 512 * sc + cols],
                                           ps[sc, h][:, :cols])
                            if sc == 0:  # m=2 has 3 frames
                                nc.scalar.mul(evt[:, 0:1], ps[sc, h][:, 0:1], 4.0 / 3.0)
                            elif sc == 3:  # m=2000,2001,2002
                                nc.scalar.mul(evt[:, 1998:1999], ps[sc, h][:, 462:463], 4.0 / 3.0)
                                nc.scalar.mul(evt[:, 1999:2000], ps[sc, h][:, 463:464], 2.0)
                                nc.scalar.mul(evt[:, 2000:2001], ps[sc, h][:, 464:465], 4.0)
                            eng = nc.sync if h == 0 else nc.gpsimd
                            eng.dma_start(
                                out[b, h, :, 512 * sc : 512 * sc + cols],
                                evt[:, 512 * sc : 512 * sc + cols],
                            )
    nc.finalize()
    return nc


def _run(inputs: dict, trace: bool = False):
    stft = np.asarray(inputs["stft_matrix"], dtype=np.float32)
    X = _prep_x(stft)                        # (16, 4, 128, UNITC)
    basis = _make_basis()
    in_maps = [
        {
            "x_in": np.ascontiguousarray(
                X[NB * c : NB * (c + 1)].reshape(4 * NB, 128, UNITC)
            ),
            "basis_in": basis,
        }
        for c in range(NCORES)
    ]
    nc = _build_nc()
    res = bass_utils.run_bass_kernel_spmd(
        nc, in_maps, core_ids=list(range(NCORES)), trace=trace
    )
    outs = []
    for c in range(NCORES):
        o = np.asarray(res.results[c]["out"])  # (NB, 2, 128, OUT_PAD) bf16
        o = o.reshape(NB, 2 * 128, OUT_PAD)[:, :, :OUT_COLS].astype(np.float32)
        outs.append(np.ascontiguousarray(o.transpose(0, 2, 1)).reshape(NB, OUT_LEN))
    return np.concatenate(outs, axis=0), res


def kernel(**inputs) -> np.ndarray:
    out, _ = _run(inputs, trace=False)
    return out


# revision 17
# speedup vs baseline: 1.2380x; 1.0220x over previous
"""Inverse STFT (nn_InverseSTFT) as a Bass/Tile kernel on 8 TRN2 NeuronCores.

Math
----
Reference: full spectrum via conjugate symmetry (F = 1024), IDFT per frame,
overlap-add with hop 256, window-sum normalize, trim n_fft//2.

Since hop = N/4, basis[f, 256j+r] = i^(fj) * basis[f, r] exactly, so
  y[256m + r] = (1/N) Re{ sum_f Z[f,m] e^(2*pi*i*f*r/N) },
  Z[f,m] = sum_{j=0..3} i^(fj) X[f, m-j].
Z preserves conjugate symmetry, so y folds to ONE real K=1024 x 256-wide
matmul per output segment (4x less PE work than matmul-per-shift).
Z is computed on the Vector engine as two shifted-add passes:
  Z2[f,c] = X[f,c] + (-1)^f X[f,c-2]
  Z [f,c] = Z2[f,c] + i^f Z2[f,c-1]
Rows are grouped by f mod 4 into 8 chunks of 128 so each pass is a plain
tensor_tensor add/subtract (uniform op per chunk, partition-aligned
Re<->Im partner chunks for odd f) — the only DVE op family with the
2-byte 2x fast mode; scalar_tensor_tensor has none and is 2x slower.
  C0: Re f=4p   C1: [Re512, Im f=4p]  C2: Re f=4p+2  C3: Im f=4p+2
  C4: Re f=4p+1 C5: Im f=4p+1         C6: Re f=4p+3  C7: Im f=4p+3
Chunks are paired into 4 "units" per batch (C0C1 / C2C3 / C4C5 / C6C7);
units with a uniform op run as one merged DVE instruction (chunk-boundary
columns compute garbage that is never read).
Window-sum normalization: basis pre-scaled by 0.25; edge segments fixed up
(m=2: 4/3, m=2000: 4/3, m=2001: 2, m=2002: 4) on the output columns.

Performance notes (from traces):
- DRAM tensors use long contiguous rows; input split into 8 x 1MB units
  alternating across the two HWDGE queues (sync + scalar) so the first
  unit lands early and DVE starts sooner.
- The PE clock starts in a low p-state; a chained warm-up matmul storm on
  junk data during the DMA lead-in ramps it before real work arrives.
- Output DMA'd per 512-column piece right after its PSUM evacuation to
  keep the drain tail short.

Sharding: pure data parallel, 2 batches per core.
"""

import numpy as np
import ml_dtypes

import concourse.bass as bass
import concourse.mybir as mybir
from concourse.tile import TileContext
from concourse import bacc, bass_utils

N_FFT = 1024
HOP = 256
B = 16
T = 2000
NCORES = 8
NB = B // NCORES          # batches per core
TPAD = 2004               # c = t + 1, t in [-1, 2003)
UNITC = 2 * TPAD          # unit = 2 chunks of TPAD cols
OUT_COLS = 2001           # segments m = 2..2002
OUT_PAD = 2048            # padded DRAM row (4 KB bf16)
SC_SIZES = (512, 512, 512, 465)  # psum column chunks over 2001
OUT_LEN = OUT_COLS * HOP  # 512256
N_WARM = 26               # PE p-state warm-up matmuls

F32 = mybir.dt.float32
BF16 = mybir.dt.bfloat16
NP_BF16 = ml_dtypes.bfloat16
ADD = mybir.AluOpType.add
SUB = mybir.AluOpType.subtract


def _row_map():
    rows = []
    rows += [(4 * p, 0) for p in range(128)]                   # C0
    rows += [(512, 0)] + [(4 * p, 1) for p in range(1, 128)]   # C1
    rows += [(4 * p + 2, 0) for p in range(128)]               # C2
    rows += [(4 * p + 2, 1) for p in range(128)]               # C3
    rows += [(4 * p + 1, 0) for p in range(128)]               # C4
    rows += [(4 * p + 1, 1) for p in range(128)]               # C5
    rows += [(4 * p + 3, 0) for p in range(128)]               # C6
    rows += [(4 * p + 3, 1) for p in range(128)]               # C7
    return rows


def _make_basis() -> np.ndarray:
    """[128, 2048] bf16: chunk ch at cols 256*ch, Bz[k, r] with conj-sym
    fold (alpha), 1/N, and the 0.25 steady-state wss normalization."""
    r = np.arange(HOP, dtype=np.float32)
    a32 = np.float32(2.0 * np.pi / N_FFT)
    Bz = np.empty((N_FFT, HOP), np.float32)
    for k, (f, c) in enumerate(_row_map()):
        ang = (np.float32(a32 * np.float32(f)) * r).astype(np.float32)
        alpha = np.float32(1.0 if f in (0, 512) else 2.0)
        v = alpha * np.cos(ang) if c == 0 else -alpha * np.sin(ang)
        Bz[k] = v / np.float32(N_FFT) * np.float32(0.25)
    big = Bz.reshape(8, 128, HOP).transpose(1, 0, 2).reshape(128, 8 * HOP)
    return np.ascontiguousarray(big.astype(NP_BF16))


def _prep_x(stft: np.ndarray) -> np.ndarray:
    """(16,513,2000,2) f32 -> (16, 4, 128, UNITC) bf16 units, zero padded."""
    rows = _row_map()
    F = np.array([f for f, _ in rows])
    C = np.array([c for _, c in rows])
    xt = stft.transpose(0, 3, 1, 2)          # (B, 2, 513, T)
    data = xt[:, C, F, :]                    # (B, 1024, T)
    X = np.zeros((B, N_FFT, TPAD), NP_BF16)
    X[:, :, 1 : 1 + T] = data.astype(NP_BF16)
    X = X.reshape(B, 4, 2, 128, TPAD).transpose(0, 1, 3, 2, 4)
    return np.ascontiguousarray(X.reshape(B, 4, 128, UNITC))


def _build_nc() -> bass.Bass:
    nc = bacc.Bacc()
    x_in = nc.dram_tensor("x_in", [4 * NB, 128, UNITC], BF16, kind="ExternalInput")
    basis_in = nc.dram_tensor("basis_in", [128, 8 * HOP], BF16, kind="ExternalInput")
    out = nc.dram_tensor("out", [NB, 2, 128, OUT_PAD], BF16, kind="ExternalOutput")

    with TileContext(nc) as tc:
        with (
            tc.tile_pool(name="xp", bufs=1) as x_pool,
            tc.tile_pool(name="z2p", bufs=1) as z2_pool,
            tc.tile_pool(name="zp", bufs=1) as z_pool,
            tc.tile_pool(name="bp", bufs=1) as b_pool,
            tc.tile_pool(name="evp", bufs=1) as ev_pool,
            tc.tile_pool(name="ps", bufs=1, space="PSUM") as psum_pool,
        ):
            # basis + warm-up junk via gpsimd (SWDGE/Pool, otherwise idle);
            # x units alternate across the two HWDGE queues (sync, scalar).
            basis_sb = b_pool.tile([128, 8 * HOP], BF16, name="basis_sb", tag="basis")
            nc.gpsimd.dma_start(basis_sb[:, :], basis_in[:, :])
            junk = b_pool.tile([128, 512], BF16, name="junk", tag="junk")
            nc.gpsimd.memset(junk[:, :], 0.0)

            # PE p-state warm-up: chained junk matmuls, done before real work.
            ps_w = psum_pool.tile([128, 512], F32, name="psw", tag="ps3_1")
            for w in range(N_WARM):
                nc.tensor.matmul(ps_w[:, :], junk[:, :128], junk[:, :],
                                 start=(w == 0), stop=(w == N_WARM - 1))

            # all x units stream on the single sync HWDGE queue (splitting
            # across queues measured ~15% slower aggregate input); the very
            # first unit is split into 2 chunk DMAs so DVE starts earlier.
            x_sb = {}
            for b in range(NB):
                for u in range(4):
                    xt = x_pool.tile([128, UNITC], BF16, name=f"x{b}_{u}",
                                     tag=f"x{b}_{u}")
                    if b == 0 and u == 0:
                        nc.sync.dma_start(xt[:, :TPAD], x_in[0, :, :TPAD])
                        nc.sync.dma_start(xt[:, TPAD:], x_in[0, :, TPAD:])
                    else:
                        nc.sync.dma_start(xt[:, :], x_in[4 * b + u])
                    x_sb[b, u] = xt

            # DVE processes each unit to completion (z2 then z) so the PE can
            # consume it immediately.
            z_sb = {}
            for b in range(NB):
                for u in range(4):
                    z2t = z2_pool.tile([128, UNITC], BF16, name=f"z2_{u}",
                                       tag=f"z2_{u}")
                    xt = x_sb[b, u]
                    if b == 0 and u == 0:
                        # split per chunk to start right after each half-DMA
                        # (slice-level deps), shaving pipeline lead-in.
                        for i in range(2):
                            o = TPAD * i
                            nc.vector.tensor_tensor(
                                out=z2t[:, o + 2 : o + TPAD],
                                in0=xt[:, o + 2 : o + TPAD],
                                in1=xt[:, o : o + TPAD - 2],
                                op=ADD,
                            )
                    else:
                        # merged Z2 over both chunks; boundary cols unread
                        nc.vector.tensor_tensor(
                            out=z2t[:, 2:UNITC],
                            in0=xt[:, 2:UNITC],
                            in1=xt[:, 0 : UNITC - 2],
                            op=ADD if u < 2 else SUB,
                        )
                    zt = z_pool.tile([128, UNITC], BF16, name=f"z{b}_{u}",
                                     tag=f"z{b}_{u}")
                    if u < 2:
                        # C0C1: Z = Z2[c] + Z2[c-1]; C2C3: minus. Merged:
                        # shift-by-1 stays chunk-aligned across the unit.
                        nc.vector.tensor_tensor(
                            out=zt[:, 3:UNITC],
                            in0=z2t[:, 3:UNITC],
                            in1=z2t[:, 2 : UNITC - 1],
                            op=ADD if u == 0 else SUB,
                        )
                    else:
                        # odd f: partner chunk Re<->Im within the unit
                        reo, imo = 0, TPAD
                        op_re, op_im = (SUB, ADD) if u == 2 else (ADD, SUB)
                        nc.vector.tensor_tensor(
                            out=zt[:, reo + 3 : reo + TPAD],
                            in0=z2t[:, reo + 3 : reo + TPAD],
                            in1=z2t[:, imo + 2 : imo + TPAD - 1],
                            op=op_re,
                        )
                        nc.vector.tensor_tensor(
                            out=zt[:, imo + 3 : imo + TPAD],
                            in0=z2t[:, imo + 3 : imo + TPAD],
                            in1=z2t[:, reo + 2 : reo + TPAD - 1],
                            op=op_im,
                        )
                    z_sb[b, u] = zt

            for b in range(NB):
                ev = {}
                for h in range(2):
                    ev[h] = ev_pool.tile([128, OUT_PAD], BF16, name=f"ev{b}_{h}",
                                         tag=f"ev{b}_{h}")
                # arrival-interleaved matmul emission: each unit's 2 chunks
                # contribute to all 8 PSUM groups as soon as its Z is ready,
                # so the PE streams behind the DVE instead of waiting for the
                # whole batch. In the last unit's block each group closes
                # (stop) and is immediately evacuated + DMA'd, so the drain
                # tail is one group, not eight.
                ps = {}
                for u in range(4):
                    for sc in range(4):
                        cols = SC_SIZES[sc]
                        for h in range(2):
                            if u == 0:
                                ps[sc, h] = psum_pool.tile(
                                    [128, 512], F32, name="ps", tag=f"ps{sc}_{h}")
                            for i in range(2):
                                ch = 2 * u + i
                                o = TPAD * i + 3 + 512 * sc
                                nc.tensor.matmul(
                                    ps[sc, h][:, :cols],
                                    basis_sb[:, HOP * ch + 128 * h : HOP * ch + 128 * h + 128],
                                    z_sb[b, u][:, o : o + cols],
                                    start=(ch == 0),
                                    stop=(ch == 7),
                                )
                            if u < 3:
                                continue
                            evt = ev[h]
                            # last batch: h=1 evacuations go to the Vector
                            # engine (idle once its Z work ends) so the drain
                            # is two parallel chains instead of one ACT chain.
                            on_dve = (b == NB - 1 and h == 1)
                            cp = nc.vector.tensor_copy if on_dve else nc.scalar.copy
                            cp(evt[:, 512 * sc : 512 * sc + cols],
                               ps[sc, h][:, :cols])
                            fixups = []
                            if sc == 0:  # m=2 has 3 frames
                                fixups = [(0, 0, 4.0 / 3.0)]
                            elif sc == 3:  # m=2000,2001,2002
                                fixups = [(1998, 462, 4.0 / 3.0),
                                          (1999, 463, 2.0), (2000, 464, 4.0)]
                            for ec, pc, s in fixups:
                                if on_dve:
                                    nc.vector.tensor_scalar_mul(
                                        evt[:, ec : ec + 1],
                                        ps[sc, h][:, pc : pc + 1], s)
                                else:
                                    nc.scalar.mul(evt[:, ec : ec + 1],
                                                  ps[sc, h][:, pc : pc + 1], s)
                            eng = nc.sync if h == 0 else nc.gpsimd
                            eng.dma_start(
                                out[b, h, :, 512 * sc : 512 * sc + cols],
                                evt[:, 512 * sc : 512 * sc + cols],
                            )
    nc.finalize()
    return nc


def _run(inputs: dict, trace: bool = False):
    stft = np.asarray(inputs["stft_matrix"], dtype=np.float32)
    X = _prep_x(stft)                        # (16, 4, 128, UNITC)
    basis = _make_basis()
    in_maps = [
        {
            "x_in": np.ascontiguousarray(
                X[NB * c : NB * (c + 1)].reshape(4 * NB, 128, UNITC)
            ),
            "basis_in": basis,
        }
        for c in range(NCORES)
    ]
    nc = _build_nc()
    res = bass_utils.run_bass_kernel_spmd(
        nc, in_maps, core_ids=list(range(NCORES)), trace=trace
    )
    outs = []
    for c in range(NCORES):
        o = np.asarray(res.results[c]["out"])  # (NB, 2, 128, OUT_PAD) bf16
        o = o.reshape(NB, 2 * 128, OUT_PAD)[:, :, :OUT_COLS].astype(np.float32)
        outs.append(np.ascontiguousarray(o.transpose(0, 2, 1)).reshape(NB, OUT_LEN))
    return np.concatenate(outs, axis=0), res


def kernel(**inputs) -> np.ndarray:
    out, _ = _run(inputs, trace=False)
    return out
